# revision 8
# baseline (speedup 1.0000x reference)
"""Trainium2 Bass kernel for nn_DeepSeekMoE_6777458393401.

Reference computation (B=8, S=2048, IN=512, H=4096, E=8, OUT=512, TOP_K=2):
    h      = x @ Wi^T + bi                      [B,S,H]
    logits = h @ Wr^T + br                      [B,S,E]
    idx    = top_k(softmax(logits), 2)          [B,S,2]   (E=8 experts)
    g      = take_along_axis(h, idx, axis=-1)   [B,S,2]   <- gathers h[...,e]
    a      = mean(g, -1) broadcast over H       [B,S,H]
    out    = a @ Wo^T + bo                      [B,S,OUT]

Because the gather picks *scalar* hidden components h[b,s,e] (e<8) and the
result is broadcast across the whole hidden dim, the module collapses to:

    logits[b,s,:] = x[b,s,:] @ (Wr@Wi)^T + (Wr@bi + br)        (E=8 wide)
    h8[b,s,:]     = x[b,s,:] @ Wi[:8,:]^T + bi[:8]             (8 wide)
    a2[b,s]       = sum of h8 at the top-2 logits              (scalar)
    out[b,s,:]    = a2[b,s] * (0.5*sum_h Wo[:,h]) + bo

i.e. one [B*S,512]@[512,16] GEMM, an 8-wide top-2 select, and a rank-1
outer product. Softmax is monotonic so top-k runs on raw logits.

Sharding: data-parallel over batch, 1 batch element (2048 tokens) per core.
"""

import numpy as np

B, S, IN, H, E, OUT = 8, 2048, 512, 4096, 8, 512
N_CORES = 8
P = 128                 # SBUF partitions
NT = S // P             # 16 token tiles per core
KC = IN // P            # 4 contraction chunks of 128

_CACHE = {}


def _build_nc():
    """Build the per-core Bass program (same NEFF on all 8 cores)."""
    import concourse.bacc as bacc
    import concourse.bass as bass
    import concourse.tile as tile
    from concourse import mybir

    f32 = mybir.dt.float32
    nc = bacc.Bacc("TRN2", target_bir_lowering=False, debug=False)

    xt = nc.dram_tensor("xt", [IN, S], f32, kind="ExternalInput")      # x[b].T
    w16 = nc.dram_tensor("w16", [IN, 16], f32, kind="ExternalInput")   # [Wri^T | Wi8^T]
    c16 = nc.dram_tensor("c16", [1, 16], f32, kind="ExternalInput")    # [cr | bi8]
    wsum = nc.dram_tensor("wsum", [1, OUT], f32, kind="ExternalInput")  # 0.5*Wo.sum(1)
    bov = nc.dram_tensor("bov", [1, OUT], f32, kind="ExternalInput")   # bo
    out = nc.dram_tensor("out", [S, OUT], f32, kind="ExternalOutput")

    with tile.TileContext(nc) as tc:
        with (
            tc.tile_pool(name="singles", bufs=1) as singles,
            tc.tile_pool(name="work", bufs=4) as work,
            tc.tile_pool(name="obuf", bufs=4) as obuf,
            tc.tile_pool(name="psum", bufs=4, space=bass.MemorySpace.PSUM) as psum,
        ):
            # ---- one-time loads -------------------------------------------
            # DMA order: w16 -> xt quarter0 -> tiny consts -> xt quarters 1-3
            # so compute can start after ~3us while the rest of x streams in.
            w16_sb = singles.tile([P, KC, 16], f32)
            nc.sync.dma_start(out=w16_sb[:], in_=w16.ap().rearrange("(k p) j -> p k j", p=P))

            xt_r = xt.ap().rearrange("(k p) t -> p k t", p=P)          # [128,4,2048]
            QT = 4                       # token tiles per quarter
            q = QT * P                   # 512 tokens per quarter
            xt_q = []
            for i in range(4):
                xt_q.append(singles.tile([P, KC, q], f32, name=f"xtq{i}", tag=f"xtq{i}"))
            nc.sync.dma_start(out=xt_q[0][:], in_=xt_r[:, :, 0:q])

            c_sb = singles.tile([1, 16], f32)
            nc.sync.dma_start(out=c_sb[:], in_=c16.ap())
            ones_row = singles.tile([1, P], f32)
            nc.vector.memset(ones_row[:], 1.0)

            # load rows then broadcast to 128 partitions on the idle Pool
            # engine (keeps the broadcast off the DMA bandwidth budget)
            wsum_row = singles.tile([1, OUT], f32)
            nc.sync.dma_start(out=wsum_row[:], in_=wsum.ap())
            bov_row = singles.tile([1, OUT], f32)
            nc.sync.dma_start(out=bov_row[:], in_=bov.ap())
            wsum_b = singles.tile([P, OUT], f32)
            nc.gpsimd.partition_broadcast(wsum_b[:], wsum_row[:], channels=P)
            bov_b = singles.tile([P, OUT], f32)
            nc.gpsimd.partition_broadcast(bov_b[:], bov_row[:], channels=P)

            for i in range(1, 4):
                nc.sync.dma_start(out=xt_q[i][:], in_=xt_r[:, :, i * q:(i + 1) * q])

            # ---- per token tile -------------------------------------------
            for grp in range(NT // QT):
                o_sb = obuf.tile([P, QT, OUT], f32)
                for j in range(QT):
                    t = grp * QT + j
                    g_ps = psum.tile([P, 16], f32)
                    # G[tok, 0:8] = logits, G[tok, 8:16] = h8 ; K=512 in 4 chunks
                    for k in range(KC):
                        nc.tensor.matmul(
                            g_ps[:],
                            lhsT=xt_q[grp][:, k, j * P:(j + 1) * P],  # [128K,128tok]
                            rhs=w16_sb[:, k, :],                      # [128K,16]
                            start=(k == 0),
                            stop=False,
                        )
                    # + bias row (K=1 rank-1 update: ones ⊗ c16)
                    nc.tensor.matmul(
                        g_ps[:], lhsT=ones_row[:], rhs=c_sb[:], start=False, stop=True,
                    )

                    g_sb = work.tile([P, 16], f32)
                    nc.scalar.copy(out=g_sb[:], in_=g_ps[:])

                    # top-8 sort of the 8 logits -> 2nd largest at column 1
                    top8 = work.tile([P, 8], f32)
                    nc.vector.max(out=top8[:], in_=g_sb[:, 0:8])

                    # a2 = sum over experts of (logit >= m2) * h8  (= top-2 sum)
                    junk8 = work.tile([P, 8], f32)
                    a2 = work.tile([P, 1], f32)
                    nc.vector.scalar_tensor_tensor(
                        out=junk8[:],
                        in0=g_sb[:, 0:8],
                        scalar=top8[:, 1:2],
                        in1=g_sb[:, 8:16],
                        op0=mybir.AluOpType.is_ge,
                        op1=mybir.AluOpType.mult,
                        accum_out=a2[:],
                    )

                    # out[tok,:] = a2 * (0.5*WoSum) + bo
                    nc.vector.scalar_tensor_tensor(
                        out=o_sb[:, j, :],
                        in0=wsum_b[:],
                        scalar=a2[:],
                        in1=bov_b[:],
                        op0=mybir.AluOpType.mult,
                        op1=mybir.AluOpType.add,
                    )
                # one 1MB DMA per 4 token tiles: out rows [grp*512, (grp+1)*512)
                nc.sync.dma_start(
                    out=out.ap().rearrange("(g j p) o -> p (g j) o", p=P, j=QT)[
                        :, grp * QT:(grp + 1) * QT, :
                    ],
                    in_=o_sb[:],
                )

    nc.compile()
    return nc


def _prep_inputs(x, Wi, bi, Wr, br, Wo, bo):
    """Fold weights on host (tiny: ~17 MFLOP) and build per-core in_maps."""
    f32 = np.float32
    x = np.asarray(x, f32)
    Wi = np.asarray(Wi, f32)
    bi = np.asarray(bi, f32)
    Wr = np.asarray(Wr, f32)
    br = np.asarray(br, f32)
    Wo = np.asarray(Wo, f32)
    bo = np.asarray(bo, f32)

    Wri = (Wr.astype(np.float64) @ Wi.astype(np.float64)).astype(f32)   # [E, IN]
    cr = (Wr.astype(np.float64) @ bi.astype(np.float64)).astype(f32) + br
    w16 = np.empty((IN, 16), f32)
    w16[:, 0:8] = Wri.T
    w16[:, 8:16] = Wi[0:8, :].T
    c16 = np.concatenate([cr, bi[0:8]]).reshape(1, 16).astype(f32)
    wsum = (0.5 * Wo.sum(axis=1, dtype=np.float64)).astype(f32).reshape(1, OUT)
    bov = bo.reshape(1, OUT).copy()

    shared = {"w16": w16, "c16": c16, "wsum": wsum, "bov": bov}
    in_maps = []
    for b in range(N_CORES):
        m = dict(shared)
        m["xt"] = np.ascontiguousarray(x[b].T)
        in_maps.append(m)
    return in_maps


def run(inputs, trace=False, **run_kwargs):
    """Compile (cached), run on 8 cores, gather. Returns (out, BassKernelResults)."""
    from concourse.bass_utils import run_bass_kernel_spmd

    if "nc" not in _CACHE:
        _CACHE["nc"] = _build_nc()
    nc = _CACHE["nc"]

    in_maps = _prep_inputs(**inputs)
    res = run_bass_kernel_spmd(
        nc, in_maps, core_ids=list(range(N_CORES)), trace=trace, **run_kwargs
    )
    out = np.stack([r["out"] for r in res.results], axis=0)  # [B, S, OUT]
    return out, res


def kernel(x, Wi, bi, Wr, br, Wo, bo) -> np.ndarray:
    out, _ = run(dict(x=x, Wi=Wi, bi=bi, Wr=Wr, br=br, Wo=Wo, bo=bo))
    return out
